# revision 1
# baseline (speedup 1.0000x reference)
"""Trainium2 Bass kernel for a hypernetwork-generated per-case MLP.

Math (fp32):
  h = silu(o @ Wc + bc)                        [C=64, H=256]
  w = einsum('ch,lhd->lcd', h, Ww) + bw        [L=4, C, 65536]
  b = einsum('ch,lhd->lcd', h, Wb) + bb        [L=4, C, 256]
  per-case 4-layer MLP over shared x [2048, 256] with silu + skip:
    a0 = silu(x @ W0 + b0); a1 = silu(a0 @ W1 + b1)
    a2 = silu(a1 @ W2 + b2); out = (a2 + a0) @ W3 + b3
  returns [C*N, 256]

Distribution over 8 NeuronCores:
  - weight-gen tensor-sharded over the d axis of Ww (each core owns a
    contiguous 8192-wide shard and computes w[:, all 64 cases, shard]);
  - per-layer AllToAll redistributes w so core k holds full-d weights for
    its 8 cases;
  - domain net data-parallel over cases (8 per core), activations kept
    feature-major [feat, n] in SBUF so every layer is a plain
    lhsT=W[i,o], rhs=A[i,n] matmul with no transposes;
  - all matmuls run as float32r (full-rate fp32 on the PE).
"""

import numpy as np

import concourse.bass as bass
import concourse.mybir as mybir
import concourse.tile as tile
from concourse import bacc
from concourse.bass import ts, ds
from concourse.bass_utils import run_bass_kernel_spmd

F32 = mybir.dt.float32
F32R = mybir.dt.float32r
F16 = mybir.dt.float16
AF = mybir.ActivationFunctionType

P = 128
NCORES = 8
C = 64          # total cases
CC = C // NCORES  # cases per core
CIN = 64        # caseNN input dim
H = 256         # caseNN hidden
HB = H // P     # h k-blocks (2)
DIN = 256       # domain feature dim (in = out = 256 for every layer)
IB = DIN // P   # 2
NL = 4          # layers
N = 2048        # samples
D = DIN * DIN   # 65536 flattened per-layer weight
DSH = D // NCORES  # 8192 per-core d shard
NCH = 4         # 512-wide chunks of N
_nc_cache = {}


def _build():
    nc = bacc.Bacc("TRN2", target_bir_lowering=False, debug=False, num_devices=NCORES)

    # ---- per-core external I/O ----
    xt = nc.dram_tensor("xt", [P, IB, N], F16, kind="ExternalInput").ap()
    ot = nc.dram_tensor("ot", [P, C], F16, kind="ExternalInput").ap()
    oto = nc.dram_tensor("oto", [P, CC], F16, kind="ExternalInput").ap()
    wc = nc.dram_tensor("wc", [P, H], F16, kind="ExternalInput").ap()
    bc2 = nc.dram_tensor("bc2", [P, HB], F32, kind="ExternalInput").ap()
    wws = nc.dram_tensor("wws", [NL, H, DSH], F16, kind="ExternalInput").ap()
    wbT = nc.dram_tensor("wbT", [P, HB, NL, DIN], F16, kind="ExternalInput").ap()
    bbT = nc.dram_tensor("bbT", [P, IB, NL], F32, kind="ExternalInput").ap()
    bwT = nc.dram_tensor("bwT", [P, NL, IB, DIN], F16, kind="ExternalInput").ap()
    yt = nc.dram_tensor("yt", [CC, IB, P, N], F32, kind="ExternalOutput").ap()

    with tile.TileContext(nc) as tc:
        with (
            tc.tile_pool(name="const", bufs=1) as const,
            tc.tile_pool(name="dram", bufs=1, space="DRAM") as dram,
            tc.tile_pool(name="ww", bufs=2) as ww,
            tc.tile_pool(name="wstg", bufs=4) as wstg,
            tc.tile_pool(name="wt", bufs=4) as wtp,
            tc.tile_pool(name="act", bufs=3) as act,
        ):
            ps_ctx = tc.tile_pool(name="ps_small", bufs=2, space="PSUM")
            ps_small = ps_ctx.__enter__()
            ps_w_ctx = tc.tile_pool(name="ps_w", bufs=2, space="PSUM")
            ps_w = ps_w_ctx.__enter__()
            # ---- load constants ----
            xt_sb = const.tile([P, IB, N], F16)
            nc.sync.dma_start(xt_sb[:], xt)
            wc_sb = const.tile([P, H], F16)
            nc.sync.dma_start(wc_sb[:], wc)
            bc_sb = const.tile([P, HB], F32)
            nc.sync.dma_start(bc_sb[:], bc2)
            ot_sb = const.tile([P, C], F16)
            nc.sync.dma_start(ot_sb[:], ot)
            oto_sb = const.tile([P, CC], F16)
            nc.sync.dma_start(oto_sb[:], oto)
            wbT_sb = const.tile([P, HB, NL, DIN], F16)
            nc.sync.dma_start(wbT_sb[:], wbT)
            bbT_sb = const.tile([P, IB, NL], F32)
            nc.sync.dma_start(bbT_sb[:], bbT)
            bwT_sb = const.tile([P, NL, IB, DIN], F16)
            nc.sync.dma_start(bwT_sb[:], bwT)

            # ---- caseNN hidden: hT[h, c] = silu(Wc.T @ o.T + bc) ----
            hT_sb = const.tile([P, HB, C], F16)
            hTo_sb = const.tile([P, HB, CC], F16)
            for kb in range(HB):
                ps = ps_small.tile([P, C], F32, tag="pss", name="psh")
                nc.tensor.matmul(
                    ps,
                    lhsT=wc_sb[:, ts(kb, P)],
                    rhs=ot_sb,
                    start=True,
                    stop=True,
                )
                nc.scalar.activation(hT_sb[:, kb, :], ps, AF.Silu, bias=bc_sb[:, kb : kb + 1])
                ps2 = ps_small.tile([P, C], F32, tag="pss", name="psh2")[:, :CC]
                nc.tensor.matmul(
                    ps2,
                    lhsT=wc_sb[:, ts(kb, P)],
                    rhs=oto_sb,
                    start=True,
                    stop=True,
                )
                nc.scalar.activation(hTo_sb[:, kb, :], ps2, AF.Silu, bias=bc_sb[:, kb : kb + 1])

            # ---- per-layer bias for own cases: bO[o, ob, l, c] ----
            bO_sb = const.tile([P, IB, NL, CC], F32)
            for l in range(NL):
                for ob in range(IB):
                    ps = ps_small.tile([P, C], F32, tag="pss", name="psb")[:, :CC]
                    for kb in range(HB):
                        nc.tensor.matmul(
                            ps,
                            lhsT=wbT_sb[:, kb, l, ts(ob, P)],
                            rhs=hTo_sb[:, kb, :],
                            start=(kb == 0),
                            stop=(kb == HB - 1),
                        )
                    nc.scalar.activation(
                        bO_sb[:, ob, l, :], ps, AF.Identity, bias=bbT_sb[:, ob, l : l + 1]
                    )

            # ---- weight-gen (all 64 cases, own d shard) + per-layer AllToAll ----
            w_fulls = []
            for l in range(NL):
                w_shard = dram.tile([C, DSH], F16, name=f"w_shard{l}")
                w_full = dram.tile([C, DSH], F16, name=f"w_full{l}")
                w_fulls.append(w_full)
                wws_l = wws[l].rearrange("(kb p) d -> p kb d", p=P)
                for q in range(4):  # quarters of the shard
                    wwt = ww.tile([P, HB, DSH // 4], F16, tag="wwt")
                    nc.sync.dma_start(wwt[:], wws_l[:, :, ts(q, DSH // 4)])
                    for ch in range(DSH // 4 // 512):
                        ps = ps_w.tile([C, 512], F32, tag="psw")
                        for kb in range(HB):
                            nc.tensor.matmul(
                                ps,
                                lhsT=hT_sb[:, kb, :],
                                rhs=wwt[:, kb, ts(ch, 512)],
                                start=(kb == 0),
                                stop=(kb == HB - 1),
                            )
                        stg = wstg.tile([C, 512], F16, tag="wstg")
                        nc.vector.tensor_copy(stg[:], ps)
                        nc.sync.dma_start(
                            w_shard[:, ds(q * (DSH // 4) + ch * 512, 512)], stg[:]
                        )
                nc.gpsimd.collective_compute(
                    "AllToAll",
                    mybir.AluOpType.bypass,
                    replica_groups=[list(range(NCORES))],
                    ins=[w_shard.opt()],
                    outs=[w_full.opt()],
                )

            ps_w_ctx.__exit__(None, None, None)
            ps_ctx.__exit__(None, None, None)
            ps_y_ctx = tc.tile_pool(name="ps_y", bufs=2, space="PSUM")
            ps_y = ps_y_ctx.__enter__()
            # ---- domain net, case-major ----
            # w_full[l] rows: j*CC + c_loc  (j = source core = d-shard index)
            # d global = i*256 + o, shard j covers i in [32j, 32j+32)
            wf_views = [wf.rearrange("(j c) (il o) -> j c il o", c=CC, o=DIN) for wf in w_fulls]
            for c in range(CC):
                a_prev = xt_sb
                a0 = None
                for l in range(NL):
                    wts = []
                    for ib in range(IB):
                        wt_t = wtp.tile([P, DIN], F16, tag="wt")
                        for jr in range(4):
                            j = 4 * ib + jr
                            nc.sync.dma_start(
                                wt_t[ds(32 * jr, 32), :], wf_views[l][j, c]
                            )
                        nc.vector.tensor_add(wt_t[:], wt_t[:], bwT_sb[:, l, ib, :])
                        wts.append(wt_t)
                    a_new = act.tile(
                        [P, IB, N], (F32 if l == NL - 1 else F16),
                        tag=("act0" if l == 0 else "act"),
                        bufs=(2 if l == 0 else 3), name=f"a_{c}_{l}"
                    )
                    for ob in range(IB):
                        ps = ps_y.tile([P, N], F32, tag="psy", name=f"psy_{c}_{l}_{ob}")
                        for ib in range(IB):
                            for nch in range(NCH):
                                nc.tensor.matmul(
                                    ps[:, ts(nch, 512)],
                                    lhsT=wts[ib][:, ts(ob, P)],
                                    rhs=a_prev[:, ib, ts(nch, 512)],
                                    start=(ib == 0),
                                    stop=(ib == IB - 1),
                                )
                        func = AF.Silu if l < NL - 1 else AF.Identity
                        nc.scalar.activation(
                            a_new[:, ob, :],
                            ps,
                            func,
                            bias=bO_sb[:, ob, l, c : c + 1],
                        )
                    if l == 0:
                        a0 = a_new
                    if l == 2:
                        a_sum = act.tile([P, IB, N], F16, tag="act", name=f"asum_{c}")
                        nc.vector.tensor_add(a_sum[:], a_new[:], a0[:])
                        a_new = a_sum
                    a_prev = a_new
                nc.sync.dma_start(yt[c].rearrange("ob p n -> p ob n"), a_prev[:])
            ps_y_ctx.__exit__(None, None, None)

    nc.compile()
    return nc


def _prep_inputs(x, o, Wc, bc, Ww, bw, Wb, bb):
    x = np.asarray(x, np.float32)
    o = np.asarray(o, np.float32)
    Wc = np.asarray(Wc, np.float32)
    bc = np.asarray(bc, np.float32)
    Ww = np.asarray(Ww, np.float32)
    bw = np.asarray(bw, np.float32)
    Wb = np.asarray(Wb, np.float32)
    bb = np.asarray(bb, np.float32)

    xt = np.ascontiguousarray(x.T.reshape(IB, P, N).transpose(1, 0, 2)).astype(np.float16)
    otf = np.zeros((P, C), np.float16)
    otf[:CIN, :] = o.T
    wcp = np.zeros((P, H), np.float16)
    wcp[:CIN, :] = Wc
    bc2 = np.ascontiguousarray(bc.reshape(HB, P).T)
    wbT = np.ascontiguousarray(Wb.reshape(NL, HB, P, DIN).transpose(2, 1, 0, 3)).astype(np.float16)
    bbT = np.ascontiguousarray(bb.reshape(NL, IB, P).transpose(2, 1, 0))
    bwT = np.ascontiguousarray(bw.reshape(NL, IB, P, DIN).transpose(2, 0, 1, 3)).astype(np.float16)

    in_maps = []
    for k in range(NCORES):
        in_maps.append(
            {
                "xt": xt,
                "ot": otf,
                "oto": np.ascontiguousarray(otf[:, k * CC : (k + 1) * CC]),
                "wc": wcp,
                "bc2": bc2,
                "wws": np.ascontiguousarray(Ww[:, :, k * DSH : (k + 1) * DSH]).astype(np.float16),
                "wbT": wbT,
                "bbT": bbT,
                "bwT": bwT,
            }
        )
    return in_maps


def _run(inputs, trace=False):
    if "nc" not in _nc_cache:
        _nc_cache["nc"] = _build()
    nc = _nc_cache["nc"]
    in_maps = _prep_inputs(**inputs)
    res = run_bass_kernel_spmd(
        nc, in_maps, core_ids=list(range(NCORES)), trace=trace
    )
    # yt per core: [CC, IB, P, N] -> [CC, N, IB*P] case-major
    parts = []
    for k in range(NCORES):
        ytk = res.results[k]["yt"]
        parts.append(ytk.transpose(0, 3, 1, 2).reshape(CC, N, DIN))
    out = np.concatenate(parts, axis=0).reshape(C * N, DIN)
    return out, res


def kernel(**inputs):
    out, _ = _run(inputs, trace=False)
    return out



# revision 5
# speedup vs baseline: 1.2265x; 1.2265x over previous
"""Trainium2 Bass kernel for a hypernetwork-generated per-case MLP.

Math (fp32):
  h = silu(o @ Wc + bc)                        [C=64, H=256]
  w = einsum('ch,lhd->lcd', h, Ww) + bw        [L=4, C, 65536]
  b = einsum('ch,lhd->lcd', h, Wb) + bb        [L=4, C, 256]
  per-case 4-layer MLP over shared x [2048, 256] with silu + skip:
    a0 = silu(x @ W0 + b0); a1 = silu(a0 @ W1 + b1)
    a2 = silu(a1 @ W2 + b2); out = (a2 + a0) @ W3 + b3
  returns [C*N, 256]

Distribution over 8 NeuronCores:
  - weight-gen tensor-sharded over the d axis of Ww (each core owns a
    contiguous 8192-wide shard and computes w[:, all 64 cases, shard]);
  - per-layer AllToAll redistributes w so core k holds full-d weights for
    its 8 cases;
  - domain net data-parallel over cases (8 per core), layer-major in two
    groups of 4 cases so the AllToAll latency never stalls the PE;
  - a tiny dummy collective issued first absorbs the one-time cross-core
    barrier; Ww input streams are spread across 3 DMA trigger queues.
"""

import numpy as np

import concourse.bass as bass
import concourse.mybir as mybir
import concourse.tile as tile
from concourse import bacc
from concourse.bass import ts, ds
from concourse.bass_utils import run_bass_kernel_spmd

F32 = mybir.dt.float32
F16 = mybir.dt.float16
AF = mybir.ActivationFunctionType

P = 128
NCORES = 8
C = 64          # total cases
CC = C // NCORES  # cases per core
CIN = 64        # caseNN input dim
H = 256         # caseNN hidden
HB = H // P     # h k-blocks (2)
DIN = 256       # domain feature dim (in = out = 256 for every layer)
IB = DIN // P   # 2
NL = 4          # layers
N = 2048        # samples
D = DIN * DIN   # 65536 flattened per-layer weight
DSH = D // NCORES  # 8192 per-core d shard
QW = DSH // 4   # 2048-wide quarter of the shard
GRP = 4         # domain case-group size
_nc_cache = {}


def _build():
    nc = bacc.Bacc("TRN2", target_bir_lowering=False, debug=False, num_devices=NCORES)

    # ---- per-core external I/O ----
    xt = nc.dram_tensor("xt", [P, IB, N], F16, kind="ExternalInput").ap()
    ot = nc.dram_tensor("ot", [P, C], F16, kind="ExternalInput").ap()
    oto = nc.dram_tensor("oto", [P, CC], F16, kind="ExternalInput").ap()
    wc = nc.dram_tensor("wc", [P, H], F16, kind="ExternalInput").ap()
    bc2 = nc.dram_tensor("bc2", [P, HB], F32, kind="ExternalInput").ap()
    wws = nc.dram_tensor("wws", [NL, H, DSH], F16, kind="ExternalInput").ap()
    wbT = nc.dram_tensor("wbT", [P, HB, NL, DIN], F16, kind="ExternalInput").ap()
    bbT = nc.dram_tensor("bbT", [P, IB, NL], F32, kind="ExternalInput").ap()
    bwT = nc.dram_tensor("bwT", [P, NL, IB, DIN], F16, kind="ExternalInput").ap()
    yt = nc.dram_tensor("yt", [CC, IB, P, N], F16, kind="ExternalOutput").ap()

    # DMA trigger queues to spread the 16MB wws stream
    def wws_engine(i):
        return (nc.sync, nc.scalar)[i % 2]

    with tile.TileContext(nc) as tc:
        with (
            tc.tile_pool(name="const", bufs=1) as const,
            tc.tile_pool(name="dram", bufs=1, space="DRAM") as dram,
            tc.tile_pool(name="ww", bufs=4) as ww,
            tc.tile_pool(name="wstg", bufs=3) as wstg,
            tc.tile_pool(name="wt", bufs=12) as wtp,
            tc.tile_pool(name="act", bufs=12) as act,
            tc.tile_pool(name="ps_small", bufs=2, space="PSUM") as ps_small,
            tc.tile_pool(name="ps_dom", bufs=3, space="PSUM") as ps_dom,
        ):
            # ---- dummy first collective: absorbs the one-time barrier ----
            dummy_in = dram.tile([NCORES, 1], F32, name="dummy_in")
            dummy_out = dram.tile([NCORES, 1], F32, name="dummy_out")
            nc.gpsimd.collective_compute(
                "AllToAll",
                mybir.AluOpType.bypass,
                replica_groups=[list(range(NCORES))],
                ins=[dummy_in.opt()],
                outs=[dummy_out.opt()],
            )

            # ---- load constants (sync queue) ----
            xt_sb = const.tile([P, IB, N], F16)
            nc.sync.dma_start(xt_sb[:], xt)
            wc_sb = const.tile([P, H], F16)
            nc.sync.dma_start(wc_sb[:], wc)
            bc_sb = const.tile([P, HB], F32)
            nc.sync.dma_start(bc_sb[:], bc2)
            ot_sb = const.tile([P, C], F16)
            nc.sync.dma_start(ot_sb[:], ot)
            oto_sb = const.tile([P, CC], F16)
            nc.sync.dma_start(oto_sb[:], oto)
            wbT_sb = const.tile([P, HB, NL, DIN], F16)
            nc.sync.dma_start(wbT_sb[:], wbT)
            bbT_sb = const.tile([P, IB, NL], F32)
            nc.sync.dma_start(bbT_sb[:], bbT)
            bwT_sb = const.tile([P, NL, IB, DIN], F16)
            nc.sync.dma_start(bwT_sb[:], bwT)

            # ---- wws loads: all 16 quarters up-front on 3 queues ----
            wws_tiles = []
            for l in range(NL):
                wws_l = wws[l].rearrange("(kb p) d -> p kb d", p=P)
                for q in range(4):
                    wwt = ww.tile([P, HB, QW], F16, tag="wwt", name=f"wwt_{l}_{q}")
                    eng = wws_engine(l * 4 + q)
                    eng.dma_start(wwt[:], wws_l[:, :, ts(q, QW)])
                    wws_tiles.append(wwt)

            # ---- caseNN hidden: hT[h, c] = silu(Wc.T @ o.T + bc) ----
            hT_sb = const.tile([P, HB, C], F16)
            hTo_sb = const.tile([P, HB, CC], F16)
            for kb in range(HB):
                ps = ps_small.tile([P, 512], F32, tag="pss", name=f"psh{kb}")
                nc.tensor.matmul(
                    ps[:, :C],
                    lhsT=wc_sb[:, ts(kb, P)],
                    rhs=ot_sb,
                    start=True,
                    stop=True,
                )
                nc.scalar.activation(hT_sb[:, kb, :], ps[:, :C], AF.Silu, bias=bc_sb[:, kb : kb + 1])
                ps2 = ps_small.tile([P, 512], F32, tag="pss", name=f"psh2{kb}")
                nc.tensor.matmul(
                    ps2[:, :CC],
                    lhsT=wc_sb[:, ts(kb, P)],
                    rhs=oto_sb,
                    start=True,
                    stop=True,
                )
                nc.scalar.activation(hTo_sb[:, kb, :], ps2[:, :CC], AF.Silu, bias=bc_sb[:, kb : kb + 1])

            # ---- per-layer bias for own cases: bO[o, ob, l, c] ----
            bO_sb = const.tile([P, IB, NL, CC], F32)
            for l in range(NL):
                for ob in range(IB):
                    ps = ps_small.tile([P, 512], F32, tag="pss", name=f"psb{l}{ob}")
                    for kb in range(HB):
                        nc.tensor.matmul(
                            ps[:, :CC],
                            lhsT=wbT_sb[:, kb, l, ts(ob, P)],
                            rhs=hTo_sb[:, kb, :],
                            start=(kb == 0),
                            stop=(kb == HB - 1),
                        )
                    nc.scalar.activation(
                        bO_sb[:, ob, l, :], ps[:, :CC], AF.Identity, bias=bbT_sb[:, ob, l : l + 1]
                    )

            # ---- weight-gen (all 64 cases, own d shard) + per-layer AllToAll ----
            w_fulls = []
            for l in range(NL):
                w_shard = dram.tile([C, DSH], F16, name=f"w_shard{l}")
                w_full = dram.tile([C, DSH], F16, name=f"w_full{l}")
                w_fulls.append(w_full)
                for q in range(4):
                    wwt = wws_tiles[l * 4 + q]
                    stg = wstg.tile([C, QW], F16, tag="wstg")
                    for ch in range(QW // 512):
                        ps = ps_small.tile([P, 512], F32, tag="pss", name=f"psw{l}{q}{ch}")
                        for kb in range(HB):
                            nc.tensor.matmul(
                                ps[:C, :],
                                lhsT=hT_sb[:, kb, :],
                                rhs=wwt[:, kb, ts(ch, 512)],
                                start=(kb == 0),
                                stop=(kb == HB - 1),
                            )
                        nc.vector.tensor_copy(stg[:, ts(ch, 512)], ps[:C, :])
                    nc.gpsimd.dma_start(w_shard[:, ts(q, QW)], stg[:])
                nc.gpsimd.collective_compute(
                    "AllToAll",
                    mybir.AluOpType.bypass,
                    replica_groups=[list(range(NCORES))],
                    ins=[w_shard.opt()],
                    outs=[w_full.opt()],
                )

            # ---- domain net, layer-major over case-groups ----
            # w_full[l] rows: j*CC + c_loc  (j = source core = d-shard index)
            # d global = i*256 + o, shard j covers i in [32j, 32j+32)
            wf_views = [wf.rearrange("(j c) (il o) -> j c il o", c=CC, o=DIN) for wf in w_fulls]
            a_cur = [None] * CC   # current input activation per case
            a_skip = [None] * CC  # skip (a0) per case
            for g in range(CC // GRP):
                cases = range(g * GRP, (g + 1) * GRP)
                for l in range(NL):
                    for c in cases:
                        wts = []
                        for ib in range(IB):
                            wt_t = wtp.tile([P, DIN], F16, tag="wt")
                            for jr in range(4):
                                j = 4 * ib + jr
                                nc.sync.dma_start(
                                    wt_t[ds(32 * jr, 32), :], wf_views[l][j, c]
                                )
                            nc.vector.tensor_add(wt_t[:], wt_t[:], bwT_sb[:, l, ib, :])
                            wts.append(wt_t)
                        a_prev = xt_sb if l == 0 else a_cur[c]
                        a_new = act.tile([P, IB, N], F16, tag="act", name=f"a_{c}_{l}")
                        func = AF.Silu if l < NL - 1 else AF.Identity
                        for ob in range(IB):
                            for hh in range(2):
                                ps = ps_dom.tile([P, 1024], F32, tag="psd", name=f"psd_{c}_{l}_{ob}_{hh}")
                                for t in range(2):
                                    nchunk = 2 * hh + t
                                    for ib in range(IB):
                                        nc.tensor.matmul(
                                            ps[:, ts(t, 512)],
                                            lhsT=wts[ib][:, ts(ob, P)],
                                            rhs=a_prev[:, ib, ts(nchunk, 512)],
                                            start=(ib == 0),
                                            stop=(ib == IB - 1),
                                        )
                                nc.scalar.activation(
                                    a_new[:, ob, ds(1024 * hh, 1024)],
                                    ps,
                                    func,
                                    bias=bO_sb[:, ob, l, c : c + 1],
                                )
                        if l == 0:
                            a_skip[c] = a_new
                        if l == 2:
                            a_sum = act.tile([P, IB, N], F16, tag="act", name=f"asum_{c}")
                            nc.vector.tensor_add(a_sum[:], a_new[:], a_skip[c][:])
                            a_new = a_sum
                        a_cur[c] = a_new
                        if l == NL - 1:
                            nc.gpsimd.dma_start(yt[c].rearrange("ob p n -> p ob n"), a_new[:])

    nc.compile()
    return nc


def _prep_inputs(x, o, Wc, bc, Ww, bw, Wb, bb):
    x = np.asarray(x, np.float32)
    o = np.asarray(o, np.float32)
    Wc = np.asarray(Wc, np.float32)
    bc = np.asarray(bc, np.float32)
    Ww = np.asarray(Ww, np.float32)
    bw = np.asarray(bw, np.float32)
    Wb = np.asarray(Wb, np.float32)
    bb = np.asarray(bb, np.float32)

    xt = np.ascontiguousarray(x.T.reshape(IB, P, N).transpose(1, 0, 2)).astype(np.float16)
    otf = np.zeros((P, C), np.float16)
    otf[:CIN, :] = o.T
    wcp = np.zeros((P, H), np.float16)
    wcp[:CIN, :] = Wc
    bc2 = np.ascontiguousarray(bc.reshape(HB, P).T)
    wbT = np.ascontiguousarray(Wb.reshape(NL, HB, P, DIN).transpose(2, 1, 0, 3)).astype(np.float16)
    bbT = np.ascontiguousarray(bb.reshape(NL, IB, P).transpose(2, 1, 0))
    bwT = np.ascontiguousarray(bw.reshape(NL, IB, P, DIN).transpose(2, 0, 1, 3)).astype(np.float16)

    in_maps = []
    for k in range(NCORES):
        in_maps.append(
            {
                "xt": xt,
                "ot": otf,
                "oto": np.ascontiguousarray(otf[:, k * CC : (k + 1) * CC]),
                "wc": wcp,
                "bc2": bc2,
                "wws": np.ascontiguousarray(Ww[:, :, k * DSH : (k + 1) * DSH]).astype(np.float16),
                "wbT": wbT,
                "bbT": bbT,
                "bwT": bwT,
            }
        )
    return in_maps


def _run(inputs, trace=False):
    if "nc" not in _nc_cache:
        _nc_cache["nc"] = _build()
    nc = _nc_cache["nc"]
    in_maps = _prep_inputs(**inputs)
    res = run_bass_kernel_spmd(
        nc, in_maps, core_ids=list(range(NCORES)), trace=trace
    )
    # yt per core: [CC, IB, P, N] f16 -> [CC, N, IB*P] case-major
    parts = []
    for k in range(NCORES):
        ytk = res.results[k]["yt"].astype(np.float32)
        parts.append(ytk.transpose(0, 3, 1, 2).reshape(CC, N, DIN))
    out = np.concatenate(parts, axis=0).reshape(C * N, DIN)
    return out, res


def kernel(**inputs):
    out, _ = _run(inputs, trace=False)
    return out


# revision 13
# speedup vs baseline: 1.2652x; 1.0315x over previous
"""v2: layer-major domain, dummy first collective, 2-queue wws, fp16 out."""

import numpy as np

import concourse.bass as bass
import concourse.mybir as mybir
import concourse.tile as tile
from concourse import bacc
from concourse.bass import ts, ds
from concourse.bass_utils import run_bass_kernel_spmd

F32 = mybir.dt.float32
F16 = mybir.dt.float16
AF = mybir.ActivationFunctionType

P = 128
NCORES = 8
C = 64
CC = C // NCORES
CIN = 64
H = 256
HB = H // P
DIN = 256
IB = DIN // P
NL = 4
N = 2048
D = DIN * DIN
DSH = D // NCORES
QW = DSH // 4
GRP = 4
_nc_cache = {}


def _build():
    nc = bacc.Bacc("TRN2", target_bir_lowering=False, debug=False, num_devices=NCORES)

    xt = nc.dram_tensor("xt", [P, IB, N], F16, kind="ExternalInput").ap()
    ot = nc.dram_tensor("ot", [P, C], F16, kind="ExternalInput").ap()
    oto = nc.dram_tensor("oto", [P, CC], F16, kind="ExternalInput").ap()
    wc = nc.dram_tensor("wc", [P, H], F16, kind="ExternalInput").ap()
    bc2 = nc.dram_tensor("bc2", [P, HB], F32, kind="ExternalInput").ap()
    wws = nc.dram_tensor("wws", [NL, H, DSH], F16, kind="ExternalInput").ap()
    wbT = nc.dram_tensor("wbT", [P, HB, NL, DIN], F16, kind="ExternalInput").ap()
    bbT = nc.dram_tensor("bbT", [P, IB, NL], F32, kind="ExternalInput").ap()
    bwT = nc.dram_tensor("bwT", [P, NL, IB, DIN], F16, kind="ExternalInput").ap()
    yt = nc.dram_tensor("yt", [CC, IB, P, N], F16, kind="ExternalOutput").ap()

    def wws_engine(i):
        return (nc.sync, nc.scalar)[i % 2]

    with tile.TileContext(nc) as tc:
        with (
            tc.tile_pool(name="const", bufs=1) as const,
            tc.tile_pool(name="dram", bufs=1, space="DRAM") as dram,
            tc.tile_pool(name="ww", bufs=4) as ww,
            tc.tile_pool(name="wstg", bufs=3) as wstg,
            tc.tile_pool(name="wt", bufs=12) as wtp,
            tc.tile_pool(name="act", bufs=12) as act,
            tc.tile_pool(name="ps_small", bufs=2, space="PSUM") as ps_small,
            tc.tile_pool(name="ps_dom", bufs=3, space="PSUM") as ps_dom,
        ):
            dummy_in = dram.tile([NCORES, 1], F32, name="dummy_in")
            dummy_out = dram.tile([NCORES, 1], F32, name="dummy_out")
            nc.gpsimd.collective_compute(
                "AllToAll",
                mybir.AluOpType.bypass,
                replica_groups=[list(range(NCORES))],
                ins=[dummy_in.opt()],
                outs=[dummy_out.opt()],
            )

            xt_sb = const.tile([P, IB, N], F16)
            nc.sync.dma_start(xt_sb[:], xt)
            wc_sb = const.tile([P, H], F16)
            nc.sync.dma_start(wc_sb[:], wc)
            bc_sb = const.tile([P, HB], F32)
            nc.sync.dma_start(bc_sb[:], bc2)
            ot_sb = const.tile([P, C], F16)
            nc.sync.dma_start(ot_sb[:], ot)
            oto_sb = const.tile([P, CC], F16)
            nc.sync.dma_start(oto_sb[:], oto)
            wbT_sb = const.tile([P, HB, NL, DIN], F16)
            nc.sync.dma_start(wbT_sb[:], wbT)
            bbT_sb = const.tile([P, IB, NL], F32)
            nc.sync.dma_start(bbT_sb[:], bbT)
            bwT_sb = const.tile([P, NL, IB, DIN], F16)
            nc.sync.dma_start(bwT_sb[:], bwT)

            wws_tiles = []
            for l in range(NL):
                wws_l = wws[l].rearrange("(kb p) d -> p kb d", p=P)
                for q in range(4):
                    wwt = ww.tile([P, HB, QW], F16, tag="wwt", name=f"wwt_{l}_{q}")
                    eng = wws_engine(l * 4 + q)
                    eng.dma_start(wwt[:], wws_l[:, :, ts(q, QW)])
                    wws_tiles.append(wwt)

            hT_sb = const.tile([P, HB, C], F16)
            hTo_sb = const.tile([P, HB, CC], F16)
            for kb in range(HB):
                ps = ps_small.tile([P, 512], F32, tag="pss", name=f"psh{kb}")
                nc.tensor.matmul(
                    ps[:, :C],
                    lhsT=wc_sb[:, ts(kb, P)],
                    rhs=ot_sb,
                    start=True,
                    stop=True,
                )
                nc.scalar.activation(hT_sb[:, kb, :], ps[:, :C], AF.Silu, bias=bc_sb[:, kb : kb + 1])
                ps2 = ps_small.tile([P, 512], F32, tag="pss", name=f"psh2{kb}")
                nc.tensor.matmul(
                    ps2[:, :CC],
                    lhsT=wc_sb[:, ts(kb, P)],
                    rhs=oto_sb,
                    start=True,
                    stop=True,
                )
                nc.scalar.activation(hTo_sb[:, kb, :], ps2[:, :CC], AF.Silu, bias=bc_sb[:, kb : kb + 1])

            bO_sb = const.tile([P, IB, NL, CC], F32)
            for l in range(NL):
                for ob in range(IB):
                    ps = ps_small.tile([P, 512], F32, tag="pss", name=f"psb{l}{ob}")
                    for kb in range(HB):
                        nc.tensor.matmul(
                            ps[:, :CC],
                            lhsT=wbT_sb[:, kb, l, ts(ob, P)],
                            rhs=hTo_sb[:, kb, :],
                            start=(kb == 0),
                            stop=(kb == HB - 1),
                        )
                    nc.scalar.activation(
                        bO_sb[:, ob, l, :], ps[:, :CC], AF.Identity, bias=bbT_sb[:, ob, l : l + 1]
                    )

            w_fulls = []
            for l in range(NL):
                w_shard = dram.tile([C, DSH], F16, name=f"w_shard{l}")
                w_full = dram.tile([C, DSH], F16, name=f"w_full{l}")
                w_fulls.append(w_full)
                for q in range(4):
                    wwt = wws_tiles[l * 4 + q]
                    stg = wstg.tile([C, QW], F16, tag="wstg")
                    for ch in range(QW // 512):
                        ps = ps_small.tile([P, 512], F32, tag="pss", name=f"psw{l}{q}{ch}")
                        for kb in range(HB):
                            nc.tensor.matmul(
                                ps[:C, :],
                                lhsT=hT_sb[:, kb, :],
                                rhs=wwt[:, kb, ts(ch, 512)],
                                start=(kb == 0),
                                stop=(kb == HB - 1),
                            )
                        nc.vector.tensor_copy(stg[:, ts(ch, 512)], ps[:C, :])
                    nc.gpsimd.dma_start(w_shard[:, ts(q, QW)], stg[:])
                nc.gpsimd.collective_compute(
                    "AllToAll",
                    mybir.AluOpType.bypass,
                    replica_groups=[list(range(NCORES))],
                    ins=[w_shard.opt()],
                    outs=[w_full.opt()],
                )

            wf_views = [wf.rearrange("(j c) (il o) -> j c il o", c=CC, o=DIN) for wf in w_fulls]
            a_cur = [None] * CC
            a_skip = [None] * CC
            for g in range(CC // GRP):
                cases = range(g * GRP, (g + 1) * GRP)
                for l in range(NL):
                    for c in cases:
                        wts = []
                        for ib in range(IB):
                            wt_t = wtp.tile([P, DIN], F16, tag="wt")
                            for jr in range(4):
                                j = 4 * ib + jr
                                nc.sync.dma_start(
                                    wt_t[ds(32 * jr, 32), :], wf_views[l][j, c]
                                )
                            nc.vector.tensor_add(wt_t[:], wt_t[:], bwT_sb[:, l, ib, :])
                            wts.append(wt_t)
                        a_prev = xt_sb if l == 0 else a_cur[c]
                        a_new = act.tile([P, IB, N], F16, tag="act", name=f"a_{c}_{l}")
                        func = AF.Silu if l < NL - 1 else AF.Identity
                        for ob in range(IB):
                            for hh in range(2):
                                ps = ps_dom.tile([P, 1024], F32, tag="psd", name=f"psd_{c}_{l}_{ob}_{hh}")
                                for t in range(2):
                                    nchunk = 2 * hh + t
                                    for ib in range(IB):
                                        nc.tensor.matmul(
                                            ps[:, ts(t, 512)],
                                            lhsT=wts[ib][:, ts(ob, P)],
                                            rhs=a_prev[:, ib, ts(nchunk, 512)],
                                            start=(ib == 0),
                                            stop=(ib == IB - 1),
                                        )
                                nc.scalar.activation(
                                    a_new[:, ob, ds(1024 * hh, 1024)],
                                    ps,
                                    func,
                                    bias=bO_sb[:, ob, l, c : c + 1],
                                )
                        if l == 0:
                            a_skip[c] = a_new
                        if l == 2:
                            a_sum = act.tile([P, IB, N], F16, tag="act", name=f"asum_{c}")
                            nc.vector.tensor_add(a_sum[:], a_new[:], a_skip[c][:])
                            a_new = a_sum
                        a_cur[c] = a_new
                        if l == NL - 1:
                            nc.gpsimd.dma_start(yt[c].rearrange("ob p n -> p ob n"), a_new[:])

    nc.compile()
    return nc


def _prep_inputs(x, o, Wc, bc, Ww, bw, Wb, bb):
    x = np.asarray(x, np.float32)
    o = np.asarray(o, np.float32)
    Wc = np.asarray(Wc, np.float32)
    bc = np.asarray(bc, np.float32)
    Ww = np.asarray(Ww, np.float32)
    bw = np.asarray(bw, np.float32)
    Wb = np.asarray(Wb, np.float32)
    bb = np.asarray(bb, np.float32)

    xt = np.ascontiguousarray(x.T.reshape(IB, P, N).transpose(1, 0, 2)).astype(np.float16)
    otf = np.zeros((P, C), np.float16)
    otf[:CIN, :] = o.T
    wcp = np.zeros((P, H), np.float16)
    wcp[:CIN, :] = Wc
    bc2 = np.ascontiguousarray(bc.reshape(HB, P).T)
    wbT = np.ascontiguousarray(Wb.reshape(NL, HB, P, DIN).transpose(2, 1, 0, 3)).astype(np.float16)
    bbT = np.ascontiguousarray(bb.reshape(NL, IB, P).transpose(2, 1, 0))
    bwT = np.ascontiguousarray(bw.reshape(NL, IB, P, DIN).transpose(2, 0, 1, 3)).astype(np.float16)

    in_maps = []
    for k in range(NCORES):
        in_maps.append(
            {
                "xt": xt,
                "ot": otf,
                "oto": np.ascontiguousarray(otf[:, k * CC : (k + 1) * CC]),
                "wc": wcp,
                "bc2": bc2,
                "wws": np.ascontiguousarray(Ww[:, :, k * DSH : (k + 1) * DSH]).astype(np.float16),
                "wbT": wbT,
                "bbT": bbT,
                "bwT": bwT,
            }
        )
    return in_maps


def _run(inputs, trace=False):
    if "nc" not in _nc_cache:
        _nc_cache["nc"] = _build()
    nc = _nc_cache["nc"]
    in_maps = _prep_inputs(**inputs)
    res = run_bass_kernel_spmd(
        nc, in_maps, core_ids=list(range(NCORES)), trace=trace
    )
    parts = []
    for k in range(NCORES):
        ytk = res.results[k]["yt"].astype(np.float32)
        parts.append(ytk.transpose(0, 3, 1, 2).reshape(CC, N, DIN))
    out = np.concatenate(parts, axis=0).reshape(C * N, DIN)
    return out, res


def kernel(**inputs):
    out, _ = _run(inputs, trace=False)
    return out


# revision 14
# speedup vs baseline: 1.3562x; 1.0719x over previous
"""v2: layer-major domain, dummy first collective, 2-queue wws, fp16 out."""

import numpy as np

import concourse.bass as bass
import concourse.mybir as mybir
import concourse.tile as tile
from concourse import bacc
from concourse.bass import ts, ds
from concourse.bass_utils import run_bass_kernel_spmd

F32 = mybir.dt.float32
F16 = mybir.dt.float16
AF = mybir.ActivationFunctionType

P = 128
NCORES = 8
C = 64
CC = C // NCORES
CIN = 64
H = 256
HB = H // P
DIN = 256
IB = DIN // P
NL = 4
N = 2048
D = DIN * DIN
DSH = D // NCORES
QW = DSH // 4
GRP = 4
_nc_cache = {}


def _build():
    nc = bacc.Bacc("TRN2", target_bir_lowering=False, debug=False, num_devices=NCORES)

    xt = nc.dram_tensor("xt", [P, IB, N], F16, kind="ExternalInput").ap()
    ot = nc.dram_tensor("ot", [P, C], F16, kind="ExternalInput").ap()
    oto = nc.dram_tensor("oto", [P, CC], F16, kind="ExternalInput").ap()
    wc = nc.dram_tensor("wc", [P, H], F16, kind="ExternalInput").ap()
    bc2 = nc.dram_tensor("bc2", [P, HB], F32, kind="ExternalInput").ap()
    wws = nc.dram_tensor("wws", [NL, H, DSH], F16, kind="ExternalInput").ap()
    wbT = nc.dram_tensor("wbT", [P, HB, NL, DIN], F16, kind="ExternalInput").ap()
    bbT = nc.dram_tensor("bbT", [P, IB, NL], F32, kind="ExternalInput").ap()
    bwT = nc.dram_tensor("bwT", [P, NL, IB, DIN], F16, kind="ExternalInput").ap()
    yt = nc.dram_tensor("yt", [CC, IB, P, N], F16, kind="ExternalOutput").ap()

    def wws_engine(i):
        return (nc.sync, nc.scalar)[i % 2]

    with tile.TileContext(nc) as tc:
        with (
            tc.tile_pool(name="const", bufs=1) as const,
            tc.tile_pool(name="dram", bufs=1, space="DRAM") as dram,
            tc.tile_pool(name="ww", bufs=4) as ww,
            tc.tile_pool(name="wstg", bufs=3) as wstg,
            tc.tile_pool(name="wt", bufs=12) as wtp,
            tc.tile_pool(name="act", bufs=12) as act,
            tc.tile_pool(name="ps", bufs=2, space="PSUM") as psp,
        ):
            dummy_in = dram.tile([NCORES, 1], F32, name="dummy_in")
            dummy_out = dram.tile([NCORES, 1], F32, name="dummy_out")
            nc.gpsimd.collective_compute(
                "AllToAll",
                mybir.AluOpType.bypass,
                replica_groups=[list(range(NCORES))],
                ins=[dummy_in.opt()],
                outs=[dummy_out.opt()],
            )

            xt_sb = const.tile([P, IB, N], F16)
            nc.sync.dma_start(xt_sb[:], xt)
            wc_sb = const.tile([P, H], F16)
            nc.sync.dma_start(wc_sb[:], wc)
            bc_sb = const.tile([P, HB], F32)
            nc.sync.dma_start(bc_sb[:], bc2)
            ot_sb = const.tile([P, C], F16)
            nc.sync.dma_start(ot_sb[:], ot)
            oto_sb = const.tile([P, CC], F16)
            nc.sync.dma_start(oto_sb[:], oto)
            wbT_sb = const.tile([P, HB, NL, DIN], F16)
            nc.sync.dma_start(wbT_sb[:], wbT)
            bbT_sb = const.tile([P, IB, NL], F32)
            nc.sync.dma_start(bbT_sb[:], bbT)
            bwT_sb = const.tile([P, NL, IB, DIN], F16)
            nc.sync.dma_start(bwT_sb[:], bwT)

            wws_tiles = []
            for l in range(NL):
                wws_l = wws[l].rearrange("(kb p) d -> p kb d", p=P)
                for q in range(4):
                    wwt = ww.tile([P, HB, QW], F16, tag="wwt", name=f"wwt_{l}_{q}")
                    eng = wws_engine(l * 4 + q)
                    eng.dma_start(wwt[:], wws_l[:, :, ts(q, QW)])
                    wws_tiles.append(wwt)

            hT_sb = const.tile([P, HB, C], F16)
            hTo_sb = const.tile([P, HB, CC], F16)
            for kb in range(HB):
                ps = psp.tile([P, 2048], F32, tag="ps", name=f"psh{kb}")
                nc.tensor.matmul(
                    ps[:, :C],
                    lhsT=wc_sb[:, ts(kb, P)],
                    rhs=ot_sb,
                    start=True,
                    stop=True,
                )
                nc.scalar.activation(hT_sb[:, kb, :], ps[:, :C], AF.Silu, bias=bc_sb[:, kb : kb + 1])
                ps2 = psp.tile([P, 2048], F32, tag="ps", name=f"psh2{kb}")
                nc.tensor.matmul(
                    ps2[:, :CC],
                    lhsT=wc_sb[:, ts(kb, P)],
                    rhs=oto_sb,
                    start=True,
                    stop=True,
                )
                nc.scalar.activation(hTo_sb[:, kb, :], ps2[:, :CC], AF.Silu, bias=bc_sb[:, kb : kb + 1])

            bO_sb = const.tile([P, IB, NL, CC], F32)
            for l in range(NL):
                for ob in range(IB):
                    ps = psp.tile([P, 2048], F32, tag="ps", name=f"psb{l}{ob}")
                    for kb in range(HB):
                        nc.tensor.matmul(
                            ps[:, :CC],
                            lhsT=wbT_sb[:, kb, l, ts(ob, P)],
                            rhs=hTo_sb[:, kb, :],
                            start=(kb == 0),
                            stop=(kb == HB - 1),
                        )
                    nc.scalar.activation(
                        bO_sb[:, ob, l, :], ps[:, :CC], AF.Identity, bias=bbT_sb[:, ob, l : l + 1]
                    )

            w_fulls = []
            for l in range(NL):
                w_shard = dram.tile([C, DSH], F16, name=f"w_shard{l}")
                w_full = dram.tile([C, DSH], F16, name=f"w_full{l}")
                w_fulls.append(w_full)
                for q in range(4):
                    wwt = wws_tiles[l * 4 + q]
                    ps = psp.tile([P, 2048], F32, tag="ps", name=f"psw{l}{q}")
                    for ch in range(QW // 512):
                        for kb in range(HB):
                            nc.tensor.matmul(
                                ps[:C, ts(ch, 512)],
                                lhsT=hT_sb[:, kb, :],
                                rhs=wwt[:, kb, ts(ch, 512)],
                                start=(kb == 0),
                                stop=(kb == HB - 1),
                            )
                    stg = wstg.tile([C, QW], F16, tag="wstg")
                    nc.vector.tensor_copy(stg[:], ps[:C, :])
                    nc.gpsimd.dma_start(w_shard[:, ts(q, QW)], stg[:])
                nc.gpsimd.collective_compute(
                    "AllToAll",
                    mybir.AluOpType.bypass,
                    replica_groups=[list(range(NCORES))],
                    ins=[w_shard.opt()],
                    outs=[w_full.opt()],
                )

            wf_views = [wf.rearrange("(j c) (il o) -> j c il o", c=CC, o=DIN) for wf in w_fulls]
            a_cur = [None] * CC
            a_skip = [None] * CC
            for g in range(CC // GRP):
                cases = range(g * GRP, (g + 1) * GRP)
                for l in range(NL):
                    for c in cases:
                        wts = []
                        for ib in range(IB):
                            wt_t = wtp.tile([P, DIN], F16, tag="wt")
                            for jr in range(4):
                                j = 4 * ib + jr
                                nc.sync.dma_start(
                                    wt_t[ds(32 * jr, 32), :], wf_views[l][j, c]
                                )
                            nc.vector.tensor_add(wt_t[:], wt_t[:], bwT_sb[:, l, ib, :])
                            wts.append(wt_t)
                        a_prev = xt_sb if l == 0 else a_cur[c]
                        a_new = act.tile([P, IB, N], F16, tag="act", name=f"a_{c}_{l}")
                        func = AF.Silu if l < NL - 1 else AF.Identity
                        a_sum = (
                            act.tile([P, IB, N], F16, tag="act", name=f"asum_{c}")
                            if l == 2
                            else None
                        )
                        for ob in range(IB):
                            ps = psp.tile([P, 2048], F32, tag="ps", name=f"psd_{c}_{l}_{ob}")
                            for nchunk in range(4):
                                for ib in range(IB):
                                    nc.tensor.matmul(
                                        ps[:, ts(nchunk, 512)],
                                        lhsT=wts[ib][:, ts(ob, P)],
                                        rhs=a_prev[:, ib, ts(nchunk, 512)],
                                        start=(ib == 0),
                                        stop=(ib == IB - 1),
                                    )
                            nc.scalar.activation(
                                a_new[:, ob, :], ps, func, bias=bO_sb[:, ob, l, c : c + 1]
                            )
                            if l == 2:
                                nc.vector.tensor_add(
                                    a_sum[:, ob, :], a_new[:, ob, :], a_skip[c][:, ob, :]
                                )
                            if l == NL - 1:
                                nc.gpsimd.dma_start(yt[c, ob], a_new[:, ob, :])
                        if l == 0:
                            a_skip[c] = a_new
                        a_cur[c] = a_sum if l == 2 else a_new

    nc.compile()
    return nc


def _prep_inputs(x, o, Wc, bc, Ww, bw, Wb, bb):
    x = np.asarray(x, np.float32)
    o = np.asarray(o, np.float32)
    Wc = np.asarray(Wc, np.float32)
    bc = np.asarray(bc, np.float32)
    Ww = np.asarray(Ww, np.float32)
    bw = np.asarray(bw, np.float32)
    Wb = np.asarray(Wb, np.float32)
    bb = np.asarray(bb, np.float32)

    xt = np.ascontiguousarray(x.T.reshape(IB, P, N).transpose(1, 0, 2)).astype(np.float16)
    otf = np.zeros((P, C), np.float16)
    otf[:CIN, :] = o.T
    wcp = np.zeros((P, H), np.float16)
    wcp[:CIN, :] = Wc
    bc2 = np.ascontiguousarray(bc.reshape(HB, P).T)
    wbT = np.ascontiguousarray(Wb.reshape(NL, HB, P, DIN).transpose(2, 1, 0, 3)).astype(np.float16)
    bbT = np.ascontiguousarray(bb.reshape(NL, IB, P).transpose(2, 1, 0))
    bwT = np.ascontiguousarray(bw.reshape(NL, IB, P, DIN).transpose(2, 0, 1, 3)).astype(np.float16)

    in_maps = []
    for k in range(NCORES):
        in_maps.append(
            {
                "xt": xt,
                "ot": otf,
                "oto": np.ascontiguousarray(otf[:, k * CC : (k + 1) * CC]),
                "wc": wcp,
                "bc2": bc2,
                "wws": np.ascontiguousarray(Ww[:, :, k * DSH : (k + 1) * DSH]).astype(np.float16),
                "wbT": wbT,
                "bbT": bbT,
                "bwT": bwT,
            }
        )
    return in_maps


def _run(inputs, trace=False):
    if "nc" not in _nc_cache:
        _nc_cache["nc"] = _build()
    nc = _nc_cache["nc"]
    in_maps = _prep_inputs(**inputs)
    res = run_bass_kernel_spmd(
        nc, in_maps, core_ids=list(range(NCORES)), trace=trace
    )
    parts = []
    for k in range(NCORES):
        ytk = res.results[k]["yt"].astype(np.float32)
        parts.append(ytk.transpose(0, 3, 1, 2).reshape(CC, N, DIN))
    out = np.concatenate(parts, axis=0).reshape(C * N, DIN)
    return out, res


def kernel(**inputs):
    out, _ = _run(inputs, trace=False)
    return out


# revision 16
# speedup vs baseline: 1.3983x; 1.0311x over previous
"""v2: layer-major domain, dummy first collective, 2-queue wws, fp16 out."""

import numpy as np

import concourse.bass as bass
import concourse.mybir as mybir
import concourse.tile as tile
from concourse import bacc
from concourse.bass import ts, ds
from concourse.bass_utils import run_bass_kernel_spmd

F32 = mybir.dt.float32
F16 = mybir.dt.float16
AF = mybir.ActivationFunctionType

P = 128
NCORES = 8
C = 64
CC = C // NCORES
CIN = 64
H = 256
HB = H // P
DIN = 256
IB = DIN // P
NL = 4
N = 2048
D = DIN * DIN
DSH = D // NCORES
QW = DSH // 4
GRP = 4
_nc_cache = {}


def _build():
    nc = bacc.Bacc("TRN2", target_bir_lowering=False, debug=False, num_devices=NCORES)

    xt = nc.dram_tensor("xt", [P, IB, N], F16, kind="ExternalInput").ap()
    ot = nc.dram_tensor("ot", [P, C], F16, kind="ExternalInput").ap()
    oto = nc.dram_tensor("oto", [P, CC], F16, kind="ExternalInput").ap()
    wc = nc.dram_tensor("wc", [P, H], F16, kind="ExternalInput").ap()
    bc2 = nc.dram_tensor("bc2", [P, HB], F32, kind="ExternalInput").ap()
    wws = nc.dram_tensor("wws", [NL, H, DSH], F16, kind="ExternalInput").ap()
    wbT = nc.dram_tensor("wbT", [P, HB, NL, DIN], F16, kind="ExternalInput").ap()
    bbT = nc.dram_tensor("bbT", [P, IB, NL], F32, kind="ExternalInput").ap()
    bwT = nc.dram_tensor("bwT", [P, NL, IB, DIN], F16, kind="ExternalInput").ap()
    yt = nc.dram_tensor("yt", [CC, IB, P, N], F16, kind="ExternalOutput").ap()

    def wws_engine(i):
        return (nc.sync, nc.scalar)[i % 2]

    with tile.TileContext(nc) as tc:
        with (
            tc.tile_pool(name="const", bufs=1) as const,
            tc.tile_pool(name="dram", bufs=1, space="DRAM") as dram,
            tc.tile_pool(name="ww", bufs=4) as ww,
            tc.tile_pool(name="wstg", bufs=3) as wstg,
            tc.tile_pool(name="wt", bufs=12) as wtp,
            tc.tile_pool(name="act", bufs=12) as act,
            tc.tile_pool(name="ps", bufs=2, space="PSUM") as psp,
        ):
            dummy_in = dram.tile([NCORES, 1], F32, name="dummy_in")
            dummy_out = dram.tile([NCORES, 1], F32, name="dummy_out")
            nc.gpsimd.collective_compute(
                "AllToAll",
                mybir.AluOpType.bypass,
                replica_groups=[list(range(NCORES))],
                ins=[dummy_in.opt()],
                outs=[dummy_out.opt()],
            )

            xt_sb = const.tile([P, IB, N], F16)
            nc.sync.dma_start(xt_sb[:], xt)
            wc_sb = const.tile([P, H], F16)
            nc.sync.dma_start(wc_sb[:], wc)
            bc_sb = const.tile([P, HB], F32)
            nc.sync.dma_start(bc_sb[:], bc2)
            ot_sb = const.tile([P, C], F16)
            nc.sync.dma_start(ot_sb[:], ot)
            oto_sb = const.tile([P, CC], F16)
            nc.sync.dma_start(oto_sb[:], oto)
            wbT_sb = const.tile([P, HB, NL, DIN], F16)
            nc.sync.dma_start(wbT_sb[:], wbT)
            bbT_sb = const.tile([P, IB, NL], F32)
            nc.sync.dma_start(bbT_sb[:], bbT)
            bwT_sb = const.tile([P, NL, IB, DIN], F16)
            nc.sync.dma_start(bwT_sb[:], bwT)

            wws_tiles = []
            for l in range(NL):
                wws_l = wws[l].rearrange("(kb p) d -> p kb d", p=P)
                for q in range(4):
                    wwt = ww.tile([P, HB, QW], F16, tag="wwt", name=f"wwt_{l}_{q}")
                    eng = wws_engine(l * 4 + q)
                    eng.dma_start(wwt[:], wws_l[:, :, ts(q, QW)])
                    wws_tiles.append(wwt)

            hT_sb = const.tile([P, HB, C], F16)
            hTo_sb = const.tile([P, HB, CC], F16)
            for kb in range(HB):
                ps = psp.tile([P, 2048], F32, tag="ps", name=f"psh{kb}")
                nc.tensor.matmul(
                    ps[:, :C],
                    lhsT=wc_sb[:, ts(kb, P)],
                    rhs=ot_sb,
                    start=True,
                    stop=True,
                )
                nc.scalar.activation(hT_sb[:, kb, :], ps[:, :C], AF.Silu, bias=bc_sb[:, kb : kb + 1])
                ps2 = psp.tile([P, 2048], F32, tag="ps", name=f"psh2{kb}")
                nc.tensor.matmul(
                    ps2[:, :CC],
                    lhsT=wc_sb[:, ts(kb, P)],
                    rhs=oto_sb,
                    start=True,
                    stop=True,
                )
                nc.scalar.activation(hTo_sb[:, kb, :], ps2[:, :CC], AF.Silu, bias=bc_sb[:, kb : kb + 1])

            bO_sb = const.tile([P, IB, NL, CC], F32)
            for l in range(NL):
                for ob in range(IB):
                    ps = psp.tile([P, 2048], F32, tag="ps", name=f"psb{l}{ob}")
                    for kb in range(HB):
                        nc.tensor.matmul(
                            ps[:, :CC],
                            lhsT=wbT_sb[:, kb, l, ts(ob, P)],
                            rhs=hTo_sb[:, kb, :],
                            start=(kb == 0),
                            stop=(kb == HB - 1),
                        )
                    nc.scalar.activation(
                        bO_sb[:, ob, l, :], ps[:, :CC], AF.Identity, bias=bbT_sb[:, ob, l : l + 1]
                    )

            w_fulls = []
            for l in range(NL):
                w_shard = dram.tile([C, DSH], F16, name=f"w_shard{l}")
                w_full = dram.tile([C, DSH], F16, name=f"w_full{l}")
                w_fulls.append(w_full)
                for q in range(4):
                    wwt = wws_tiles[l * 4 + q]
                    ps = psp.tile([P, 2048], F32, tag="ps", name=f"psw{l}{q}")
                    for ch in range(QW // 512):
                        for kb in range(HB):
                            nc.tensor.matmul(
                                ps[:C, ts(ch, 512)],
                                lhsT=hT_sb[:, kb, :],
                                rhs=wwt[:, kb, ts(ch, 512)],
                                start=(kb == 0),
                                stop=(kb == HB - 1),
                            )
                    stg = wstg.tile([C, QW], F16, tag="wstg")
                    nc.vector.tensor_copy(stg[:], ps[:C, :])
                    nc.gpsimd.dma_start(w_shard[:, ts(q, QW)], stg[:])
                nc.gpsimd.collective_compute(
                    "AllToAll",
                    mybir.AluOpType.bypass,
                    replica_groups=[list(range(NCORES))],
                    ins=[w_shard.opt()],
                    outs=[w_full.opt()],
                )

            wf_views = [wf.rearrange("(j c) (il o) -> j c il o", c=CC, o=DIN) for wf in w_fulls]

            def load_wt(l, c):
                wts = []
                for ib in range(IB):
                    wt_t = wtp.tile([P, DIN], F16, tag="wt")
                    eng = (nc.sync, nc.scalar)[ib]
                    eng.dma_start(wt_t[:], wf_views[l][ds(GRP * ib, GRP), c])
                    nc.vector.tensor_add(wt_t[:], wt_t[:], bwT_sb[:, l, ib, :])
                    wts.append(wt_t)
                return wts

            seq = []
            for g in range(CC // GRP):
                for l in range(NL):
                    for c in range(g * GRP, (g + 1) * GRP):
                        seq.append((l, c))

            a_cur = [None] * CC
            a_skip = [None] * CC
            wts_next = load_wt(*seq[0])
            for i, (l, c) in enumerate(seq):
                wts = wts_next
                if i + 1 < len(seq):
                    wts_next = load_wt(*seq[i + 1])
                if True:
                    if True:
                        a_prev = xt_sb if l == 0 else a_cur[c]
                        a_new = act.tile([P, IB, N], F16, tag="act", name=f"a_{c}_{l}")
                        func = AF.Silu if l < NL - 1 else AF.Identity
                        a_sum = (
                            act.tile([P, IB, N], F16, tag="act", name=f"asum_{c}")
                            if l == 2
                            else None
                        )
                        for ob in range(IB):
                            ps = psp.tile([P, 2048], F32, tag="ps", name=f"psd_{c}_{l}_{ob}")
                            for nchunk in range(4):
                                for ib in range(IB):
                                    nc.tensor.matmul(
                                        ps[:, ts(nchunk, 512)],
                                        lhsT=wts[ib][:, ts(ob, P)],
                                        rhs=a_prev[:, ib, ts(nchunk, 512)],
                                        start=(ib == 0),
                                        stop=(ib == IB - 1),
                                    )
                            nc.scalar.activation(
                                a_new[:, ob, :], ps, func, bias=bO_sb[:, ob, l, c : c + 1]
                            )
                            if l == 2:
                                nc.vector.tensor_add(
                                    a_sum[:, ob, :], a_new[:, ob, :], a_skip[c][:, ob, :]
                                )
                            if l == NL - 1:
                                nc.gpsimd.dma_start(yt[c, ob], a_new[:, ob, :])
                        if l == 0:
                            a_skip[c] = a_new
                        a_cur[c] = a_sum if l == 2 else a_new

    nc.compile()
    return nc


def _prep_inputs(x, o, Wc, bc, Ww, bw, Wb, bb):
    x = np.asarray(x, np.float32)
    o = np.asarray(o, np.float32)
    Wc = np.asarray(Wc, np.float32)
    bc = np.asarray(bc, np.float32)
    Ww = np.asarray(Ww, np.float32)
    bw = np.asarray(bw, np.float32)
    Wb = np.asarray(Wb, np.float32)
    bb = np.asarray(bb, np.float32)

    xt = np.ascontiguousarray(x.T.reshape(IB, P, N).transpose(1, 0, 2)).astype(np.float16)
    otf = np.zeros((P, C), np.float16)
    otf[:CIN, :] = o.T
    wcp = np.zeros((P, H), np.float16)
    wcp[:CIN, :] = Wc
    bc2 = np.ascontiguousarray(bc.reshape(HB, P).T)
    wbT = np.ascontiguousarray(Wb.reshape(NL, HB, P, DIN).transpose(2, 1, 0, 3)).astype(np.float16)
    bbT = np.ascontiguousarray(bb.reshape(NL, IB, P).transpose(2, 1, 0))
    bwT = np.ascontiguousarray(bw.reshape(NL, IB, P, DIN).transpose(2, 0, 1, 3)).astype(np.float16)

    in_maps = []
    for k in range(NCORES):
        in_maps.append(
            {
                "xt": xt,
                "ot": otf,
                "oto": np.ascontiguousarray(otf[:, k * CC : (k + 1) * CC]),
                "wc": wcp,
                "bc2": bc2,
                "wws": np.ascontiguousarray(Ww[:, :, k * DSH : (k + 1) * DSH]).astype(np.float16),
                "wbT": wbT,
                "bbT": bbT,
                "bwT": bwT,
            }
        )
    return in_maps


def _run(inputs, trace=False):
    if "nc" not in _nc_cache:
        _nc_cache["nc"] = _build()
    nc = _nc_cache["nc"]
    in_maps = _prep_inputs(**inputs)
    res = run_bass_kernel_spmd(
        nc, in_maps, core_ids=list(range(NCORES)), trace=trace
    )
    parts = []
    for k in range(NCORES):
        ytk = res.results[k]["yt"].astype(np.float32)
        parts.append(ytk.transpose(0, 3, 1, 2).reshape(CC, N, DIN))
    out = np.concatenate(parts, axis=0).reshape(C * N, DIN)
    return out, res


def kernel(**inputs):
    out, _ = _run(inputs, trace=False)
    return out
